# revision 15
# baseline (speedup 1.0000x reference)
"""BarrierNet Trainium2 kernel v2: tiny MLP (10->128->{32,32}->{2,1}) + halfspace
QP projection over a 524288-row batch, data-parallel over 8 NeuronCores.

The workload is activation-column bound: SiLU on the Scalar engine (ACT) runs
1 col/cycle @1.2GHz => ~82us/core floor if ACT evaluates every SiLU. v2 splits
the transcendental work across engines:
  - ACT: L1 SiLU (1 col/sample, bias fused) + alpha tanh + the biased u3
    evacuation (Copy activation with per-partition bias = L3 bias fold)
  - DVE: L2 SiLU via a custom fused 7-stage op (per-unit cubic approximation
    of sigmoid with the layer bias folded into a per-partition scalar; the L2
    preactivation range is ~+-1.7 where the unclipped cubic is accurate to
    <1e-3), plus the forward-transpose evacuation and the QP epilogue
  - GpSimd: obs fp32->bf16 pad-cast + epilogue dot-product precomputes
  - PE: 4-blocks-per-128x128 forward transposes, L1/L2/L3 matmuls with
    row/col array tiling, and the back-transpose.
PSUM (8 banks): xt 1 + l1 4 + l2-half ring1 1 + u3 ring1 1 + ut 1.
"""

import numpy as np
import ml_dtypes

B, F, H1, C = 524288, 10, 128, 2
NCORES = 8
BC = B // NCORES            # 65536 samples per core
P = 128
CPP = BC // P               # 512 samples per partition == blocks per core
NBLK = CPP
BLK_PER_CHUNK = 16
NCHUNK = NBLK // BLK_PER_CHUNK   # 32
R2 = 0.8 * 0.8
NPC = 4                     # epilogue pieces
BPP = NBLK // NPC           # 128 blocks per piece
FP = 32                     # padded feature stride
CHW = BLK_PER_CHUNK // 4    # transpose windows per chunk

_BUILT = None
_OPS = None


def _register_dve_ops():
    """Register the custom fused DVE ops (idempotent):
      SILU_CUBIC_BIAS_ANT: out = zb*(imm2 + zb*(C1 - zb^2*C3)), zb = in0+C0
      BAR_ALPHA_ANT:       out = (in0 - C0) * (1 + in1)
    """
    global _OPS
    if _OPS is not None:
        return _OPS
    import concourse.dve_ops as do
    from concourse.dve_spec import (Spec, Src0, Src1, C0, C1, C2, C3, One,
                                    sq, lower, _spill_c3_to_src1)

    def _reg(name, spec, rd1):
        for op in do.OPS:
            if op.name == name:
                return op
        do._SUB_OPCODE_FOR_NAME[name] = do._CUSTOM_DVE_ROW_BASE + len(do.OPS)
        shas = {}
        for ver in ("v3", "v4"):
            s = do.DveOpSpec(name=name, opcode=do.get_dve_sub_opcode(name),
                             uops=lower(spec, ver=ver), rd1_en=rd1)
            shas[ver] = s.sha(ver)
        op = do.DveOp(name, spec, subdim=False, uops_sha=shas)
        do.OPS.append(op)
        do.CUSTOM_DVE_SPECS[name] = spec
        return op

    def silu_ref(in0, in1, s0, s1, imm2):
        zb = np.asarray(in0, np.float32) + s0
        return (zb * (imm2 + zb * (s1 - zb * zb * in1))).astype(np.float32)

    zb = Src0 + C0
    silu_spec = Spec(body=_spill_c3_to_src1(zb * (C2 + zb * (C1 - sq(zb) * C3))),
                     reference=silu_ref)
    silu_op = _reg("SILU_CUBIC_BIAS_ANT", silu_spec, True)

    def ba_ref(in0, in1, s0, s1, imm2):
        return ((np.asarray(in0, np.float32) - s0)
                * (1.0 + np.asarray(in1, np.float32))).astype(np.float32)

    ba_spec = Spec(body=(Src0 - C0) * (One + Src1), reference=ba_ref)
    ba_op = _reg("BAR_ALPHA_ANT", ba_spec, True)

    def am_ref(in0, in1, s0, s1, imm2):
        return np.minimum(np.asarray(in0, np.float32)
                          + np.asarray(in1, np.float32), 0.0).astype(np.float32)

    from concourse.dve_spec import Zero, minn
    am_spec = Spec(body=minn(Src0 + Src1, Zero), reference=am_ref)
    am_op = _reg("ADD_MIN0_ANT", am_spec, True)

    _OPS = (silu_op, ba_op, am_op)
    return _OPS


def _legalize_single_wait(nc, mybir):
    """This walrus build encodes at most ONE sync wait per instruction; split
    multi-wait instructions into preceding NoOp wait-carriers."""
    n = 0
    for f in nc.m.functions:
        for b in f.blocks:
            new_list = []
            changed = False
            for inst in b.instructions:
                si = inst.sync_info
                if si is not None and len(si.on_wait) > 1:
                    waits = list(si.on_wait)
                    for k, w in enumerate(waits[1:]):
                        new_list.append(mybir.InstNoOp(
                            name=f"{inst.name}-wsplit-{k}", engine=inst.engine,
                            ins=[], outs=[],
                            sync_info=mybir.SyncInfo(on_update=[], on_wait=[w])))
                        n += 1
                    si.on_wait = waits[:1]
                    inst.sync_info = si
                    changed = True
                new_list.append(inst)
            if changed:
                b.instructions = new_list
    return n


def _build():
    global _BUILT
    if _BUILT is not None:
        return _BUILT
    import concourse.bass as bass
    import concourse.tile as tile
    import concourse.mybir as mybir

    silu2, ba_op, am_op = _register_dve_ops()

    f32 = mybir.dt.float32
    bf16 = mybir.dt.bfloat16
    AF = mybir.ActivationFunctionType
    ALU = mybir.AluOpType

    nc = bass.Bass("TRN2")
    obs_d = nc.dram_tensor("obs", [BC, F], f32, kind="ExternalInput")
    w1t_d = nc.dram_tensor("w1t", [P, 128], bf16, kind="ExternalInput")
    w2t_d = nc.dram_tensor("w2t", [P, 64], bf16, kind="ExternalInput")
    w3_d = nc.dram_tensor("w3", [P, 4], bf16, kind="ExternalInput")
    idb_d = nc.dram_tensor("idb", [P, 128], bf16, kind="ExternalInput")
    b1_d = nc.dram_tensor("b1v", [P, 1], f32, kind="ExternalInput")
    cb_d = nc.dram_tensor("cbv", [P, 1], f32, kind="ExternalInput")
    c1_d = nc.dram_tensor("c1v", [P, 1], f32, kind="ExternalInput")
    c3_d = nc.dram_tensor("c3v", [P, 1], f32, kind="ExternalInput")
    ba_d = nc.dram_tensor("bav", [P, 1], f32, kind="ExternalInput")
    out_d = nc.dram_tensor("out", [BC, C], f32, kind="ExternalOutput")

    obs_ap = obs_d[:].rearrange("(p c) f -> p (c f)", p=P)   # [128, 5120]
    out_ap = out_d[:].rearrange("(p c) u -> p (c u)", p=P)   # [128, 1024]

    with tile.TileContext(nc) as tc:
        from contextlib import ExitStack
        es = ExitStack()
        with es:
            cpool = es.enter_context(tc.tile_pool(name="const", bufs=1))
            bigpool = es.enter_context(tc.tile_pool(name="big", bufs=1))
            wpool = es.enter_context(tc.tile_pool(name="work", bufs=3))
            epool = es.enter_context(tc.tile_pool(name="epi", bufs=1))
            ppool = es.enter_context(tc.tile_pool(name="ps", bufs=1, space="PSUM"))

            # ---- obs (whole-core fp32; persists for the epilogue) ----
            obsf = bigpool.tile([P, NBLK * F], f32)          # [128, 5120]

            def load_piece(t):
                nc.sync.dma_start(
                    out=obsf[:, BPP * F * t:BPP * F * (t + 1)],
                    in_=obs_ap[:, BPP * F * t:BPP * F * (t + 1)])

            load_piece(0)

            # ---- constants ----
            def cdma(shape, dt, src, tag):
                t_ = cpool.tile(shape, dt, tag=tag, name=tag)
                nc.sync.dma_start(out=t_, in_=src[:])
                return t_

            idb = cdma([P, 128], bf16, idb_d, "idb")
            w1t = cdma([P, 128], bf16, w1t_d, "w1t")
            b1s = cdma([P, 1], f32, b1_d, "b1s")
            w2t = cdma([P, 64], bf16, w2t_d, "w2t")
            w3s = cdma([P, 4], bf16, w3_d, "w3s")
            cbs = cdma([P, 1], f32, cb_d, "cbs")
            c1s = cdma([P, 1], f32, c1_d, "c1s")
            c3s = cdma([P, 1], f32, c3_d, "c3s")
            bas = cdma([P, 1], f32, ba_d, "bas")

            for t in range(1, NPC):
                load_piece(t)

            # ---- persistent SBUF ----
            ubuf = bigpool.tile([P, CPP * 3], f32)        # (u0,u1,apre)+bias
            outt = bigpool.tile([P, CPP * C], f32)
            u0v = ubuf[:, 0::3]
            u1v = ubuf[:, 1::3]
            apv = ubuf[:, 2::3]

            def et(tag):
                return epool.tile([P, CPP], f32, tag=tag, name=tag)

            ss_t, lf_t, pp_t, th_t, t2_t, ggc_t, rec_t, s4_t, q2_t, w0_t, w1_t = (
                et(x) for x in ("ss", "lf", "pp", "th", "t2", "ggc", "rec",
                                "s4", "q2", "w0", "w1"))

            # ---- ring tiles (SBUF) ----
            def obsb_t(q):
                return wpool.tile([P, BLK_PER_CHUNK * FP], bf16, tag="obsb",
                                  name=f"obsb{q}")

            def xts_t(q):
                return wpool.tile([P, 512], bf16, tag="xts", name=f"xts{q}")

            def h1_t(q):
                return wpool.tile([P, 2048], bf16, tag="h1", name=f"h1{q}")

            def x2_t(q, h):
                return wpool.tile([P, 512], bf16, tag="x2", name=f"x2_{q}_{h}")

            def u3s_t(q):
                return wpool.tile([P, 512], bf16, tag="u3s", name=f"u3s{q}")

            # ---- PSUM tiles: 1+4+1+1+1 = 8 banks ----
            def xtp_t(q):
                return ppool.tile([P, 512], bf16, tag="xt", name=f"xtp{q}")

            def l1p_t(q):
                return ppool.tile([P, 2048], f32, tag="l1", name=f"l1p{q}")

            def l2p_t(q, h):
                return ppool.tile([P, 512], f32, tag="l2h", name=f"l2p{q}_{h}")

            def u3p_t(q):
                return ppool.tile([P, 512], f32, tag="u3", name=f"u3p{q}")

            def utp_t(q):
                return ppool.tile([P, 512], f32, tag="ut", name=f"utp{q}")

            state = {}

            def stC(q):
                """obs fp32 -> padded bf16 blocks (gpsimd)."""
                ob = obsb_t(q)
                obv = ob[:].rearrange("p (c f) -> p c f", f=FP)
                if q < 2:
                    nc.gpsimd.memset(obv[:, :, F:FP], 0.0)
                src = obsf[:, BLK_PER_CHUNK * F * q:BLK_PER_CHUNK * F * (q + 1)]
                nc.gpsimd.tensor_copy(
                    out=obv[:, :, 0:F],
                    in_=src.rearrange("p (c f) -> p c f", f=F))
                state[('obsb', q)] = ob

            def stT(q):
                """PE fwd transpose: 4 windows of 4 padded blocks."""
                ob = state.pop(('obsb', q))
                xtp = xtp_t(q)
                for w in range(CHW):
                    nc.tensor.transpose(
                        out=xtp[:, 128 * w:128 * (w + 1)],
                        in_=ob[:, 128 * w:128 * (w + 1)],
                        identity=idb[:])
                state[('xtp', q)] = xtp

            def stE(q):
                """evac xtp(f32 psum) -> xts(bf16 sbuf) on DVE."""
                xtp = state.pop(('xtp', q))
                xts = xts_t(q)
                nc.vector.tensor_copy(out=xts[:], in_=xtp[:])
                state[('xts', q)] = xts

            def stL1(q):
                xts = state.pop(('xts', q))
                l1p = l1p_t(q)
                for j in range(4):
                    nc.tensor.matmul(
                        out=l1p[:, 512 * j:512 * (j + 1)],
                        lhsT=w1t[32 * j:32 * j + F, :],
                        rhs=xts[32 * j:32 * j + F, :],
                        tile_position=(32 * j, 0))
                state[('l1p', q)] = l1p

            def stS1(q):
                """L1 silu fully on ACT (bias fused)."""
                l1p = state.pop(('l1p', q))
                h1 = h1_t(q)
                nc.scalar.activation(out=h1[:], in_=l1p[:], func=AF.Silu,
                                     bias=b1s[:, 0:1], scale=1.0)
                state[('h1', q)] = h1

            def stL2(q, h):
                """half-chunk L2: groups g = 2h, 2h+1 share l2 column range."""
                h1 = state[('h1', q)]
                l2p = l2p_t(q, h)
                for g in (2 * h, 2 * h + 1):
                    pb = 64 * (g % 2)
                    nc.tensor.matmul(
                        out=l2p[pb:pb + 64, :],
                        lhsT=w2t[:],
                        rhs=h1[:, 512 * g:512 * (g + 1)],
                        tile_position=(0, pb))
                state[('l2p', q, h)] = l2p

            def stS2(q, h):
                """L2 silu on DVE via custom cubic (bias folded into s0)."""
                l2p = state.pop(('l2p', q, h))
                x2 = x2_t(q, h)
                nc.vector._custom_dve(
                    silu2, out=x2[:], in0=l2p[:], in1=c3s[:, 0:1],
                    s0=cbs[:, 0:1], s1=c1s[:, 0:1], imm2=0.5)
                state[('x2', q, h)] = x2

            def stL3(q, h):
                x2 = state.pop(('x2', q, h))
                u3p = state[('u3p', q)]
                for g in (2 * h, 2 * h + 1):
                    pb = 64 * (g % 2)
                    nc.tensor.matmul(
                        out=u3p[32 * g:32 * g + 3, :],
                        lhsT=w3s[pb:pb + 64, 0:3],
                        rhs=x2[pb:pb + 64, :],
                        tile_position=(pb, 32 * g))

            def stC3(q):
                """u3p(f32 psum) -> u3s(bf16 sbuf) + per-partition L3 bias
                (ACT Copy activation)."""
                u3p = state.pop(('u3p', q))
                u3s = u3s_t(q)
                nc.scalar.activation(out=u3s[:], in_=u3p[:], func=AF.Identity,
                                     bias=bas[:, 0:1], scale=1.0)
                state[('u3s', q)] = u3s

            def stBT(q):
                u3s = state.pop(('u3s', q))
                utp = utp_t(q)
                for w in range(4):
                    nc.tensor.matmul(
                        out=utp[:, 128 * w:128 * (w + 1)],
                        lhsT=u3s[:, 128 * w:128 * (w + 1)],
                        rhs=idb[:])
                state[('utp', q)] = utp

            def stX(q):
                """extract utp -> ubuf (u0,u1,ap interleaved, biased)."""
                utp = state.pop(('utp', q))
                src = utp[:].rearrange("p (w g x) -> p w g x",
                                       w=4, g=4)[:, :, :, 0:3]
                dst = ubuf[:, 48 * q:48 * (q + 1)].rearrange(
                    "p (w g f) -> p w g f", w=4, g=4)
                nc.scalar.copy(out=dst, in_=src)

            def pre_sslf(t):
                """early per-piece precompute from obsf only (gpsimd)."""
                sl = slice(BPP * t, BPP * (t + 1))
                rx = obsf[:, 6::F][:, sl]
                ry = obsf[:, 7::F][:, sl]
                vx = obsf[:, 8::F][:, sl]
                vy = obsf[:, 9::F][:, sl]
                ss, lf, w0, w1 = (x[:, sl] for x in (ss_t, lf_t, w0_t, w1_t))
                GT = nc.gpsimd.tensor_tensor
                GT(out=ss, in0=rx, in1=rx, op=ALU.mult)
                GT(out=w0, in0=ry, in1=ry, op=ALU.mult)
                GT(out=ss, in0=ss, in1=w0, op=ALU.add)
                GT(out=w0, in0=rx, in1=vx, op=ALU.mult)
                GT(out=w1, in0=ry, in1=vy, op=ALU.mult)
                GT(out=lf, in0=w0, in1=w1, op=ALU.add)

            def pre_rec():
                """whole-core ggc + fast reciprocal (DVE), once."""
                nc.vector.tensor_scalar(out=ggc_t[:], in0=ss_t[:], scalar1=4.0,
                                        scalar2=1e-12, op0=ALU.mult,
                                        op1=ALU.max)
                nc.vector.reciprocal_approx_fast(out=rec_t[:], in_=ggc_t[:])

            def epi(t):
                """QP epilogue for piece t (blocks [128t,128t+128))."""
                c0, c1 = BPP * t, BPP * (t + 1)
                sl = slice(c0, c1)
                rx = obsf[:, 6::F][:, sl]
                ry = obsf[:, 7::F][:, sl]
                u0, u1, ap_ = u0v[:, sl], u1v[:, sl], apv[:, sl]
                ss, lf, pp, th, t2, rec, q2, w0, w1 = (
                    x[:, sl] for x in (ss_t, lf_t, pp_t, th_t, t2_t,
                                       rec_t, q2_t, w0_t, w1_t))
                GT = nc.gpsimd.tensor_tensor
                TT = nc.vector.tensor_tensor
                # gpsimd: pp = rx*u0+ry*u1 - lf
                GT(out=w0, in0=rx, in1=u0, op=ALU.mult)
                GT(out=w1, in0=ry, in1=u1, op=ALU.mult)
                GT(out=pp, in0=w0, in1=w1, op=ALU.add)
                GT(out=pp, in0=pp, in1=lf, op=ALU.subtract)
                # ACT: th = tanh(ap/2)   (b32 pre-added via the C3 bias)
                nc.scalar.activation(out=th, in_=ap_, func=AF.Tanh, scale=0.5)
                # DVE: t2 = (ss - R2)*(1+th); q2 = min(pp + t2, 0) * rec
                nc.vector._custom_dve(ba_op, out=t2, in0=ss, in1=th, s0=R2)
                nc.vector._custom_dve(am_op, out=q2, in0=pp, in1=t2)
                TT(out=q2, in0=q2, in1=rec, op=ALU.mult)
                TT(out=w0, in0=q2, in1=rx, op=ALU.mult)
                nc.vector.scalar_tensor_tensor(out=outt[:, 0::2][:, sl],
                                               in0=w0, scalar=-4.0, in1=u0,
                                               op0=ALU.mult, op1=ALU.add)
                TT(out=w1, in0=q2, in1=ry, op=ALU.mult)
                nc.vector.scalar_tensor_tensor(out=outt[:, 1::2][:, sl],
                                               in0=w1, scalar=-4.0, in1=u1,
                                               op0=ALU.mult, op1=ALU.add)
                nc.sync.dma_start(out=out_ap[:, 2 * c0:2 * c1],
                                  in_=outt[:, 2 * c0:2 * c1])

            # ---- skewed pipeline ----
            def phase1(q):
                stC(q); stT(q); stE(q)

            def phase2(q):
                stL1(q); stS1(q)

            def phase3(q):
                state[('u3p', q)] = u3p_t(q)
                stL2(q, 0); stS2(q, 0); stL2(q, 1); stS2(q, 1)
                stL3(q, 0); stL3(q, 1)
                state.pop(('h1', q))
                stC3(q)

            def phase4(q):
                stBT(q); stX(q)
                if (q + 1) % (NCHUNK // NPC) == 0:
                    epi((q + 1) // (NCHUNK // NPC) - 1)

            def warm_burst(k):
                # back-to-back matmuls to fire the PE HAM clock-gate
                warm = ppool.tile([P, 512], f32, tag="ut", name=f"warm{k}")
                for w in range(40):
                    nc.tensor.matmul(out=warm[:, 0:128], lhsT=idb[:],
                                     rhs=idb[:])

            def fill(warm, n):
                for _ in range(n):
                    nc.tensor.matmul(out=warm[:, 0:128], lhsT=idb[:],
                                     rhs=idb[:])

            for i in range(NCHUNK + 3):
                if i in (4, 9):
                    warm_burst(i)
                warm = (ppool.tile([P, 512], f32, tag="ut", name=f"wf{i}")
                        if 4 < i else None)
                if i >= 3:
                    phase4(i - 3)
                if warm is not None:
                    fill(warm, 2)
                if 2 <= i < NCHUNK + 2:
                    phase3(i - 2)
                if warm is not None:
                    fill(warm, 2)
                if 1 <= i < NCHUNK + 1:
                    phase2(i - 1)
                if warm is not None:
                    fill(warm, 2)
                if i < NCHUNK:
                    phase1(i)
                if warm is not None:
                    fill(warm, 2)
                if i < NPC:
                    pre_sslf(i)
                if i == NPC:
                    pre_rec()

    # populate .instr bytes for InstCustomDveAnt (raw bass skips this pass;
    # without it walrus codegen fails with "ISA wrong length")
    mybir.codegen_inst_isa_subclasses(nc)
    _legalize_single_wait(nc, mybir)
    _BUILT = nc
    return nc


def _fit_l2_cubic(W1, b1, W21, b21, W22, b22):
    """Per-unit sigmoid cubic (c1, c3) with c0=0.5 fixed, fit over the
    estimated z2 distribution; packed for the [u21;u22;u21;u22] layout."""
    gh_x, gh_w = np.polynomial.hermite_e.hermegauss(41)
    mu1 = b1.astype(np.float64)
    sd1 = np.linalg.norm(W1.astype(np.float64), axis=1)
    z = mu1[:, None] + sd1[:, None] * gh_x[None, :]
    h = z / (1.0 + np.exp(-z))
    wts = gh_w / gh_w.sum()
    mh = (h * wts).sum(1)
    vh = ((h - mh[:, None]) ** 2 * wts).sum(1)
    W2 = np.concatenate([W21, W22], 0).astype(np.float64)   # [64, 128]
    b2 = np.concatenate([b21, b22]).astype(np.float64)
    mu2 = W2 @ mh + b2
    sd2 = np.maximum(np.sqrt((W2 ** 2) @ vh), 0.05)
    t = np.linspace(-8.0, 8.0, 301)
    zz = mu2[:, None] + sd2[:, None] * t[None, :]
    wq = np.exp(-0.5 * t ** 2)[None, :] * np.ones((64, 1))
    y = zz / (1.0 + np.exp(-zz))
    tgt = np.where(np.abs(zz) > 1e-9,
                   y / np.where(np.abs(zz) > 1e-9, zz, 1.0), 0.5) - 0.5
    X = np.stack([zz, -zz ** 3], axis=2)
    ww = wq * zz ** 2
    A = np.einsum('ugi,ug,ugj->uij', X, ww, X) + np.eye(2)[None] * 1e-12
    bb = np.einsum('ugi,ug,ug->ui', X, ww, tgt)
    cc = np.linalg.solve(A, bb[:, :, None])[:, :, 0]

    def packv(v):
        return np.concatenate([v[:32], v[32:], v[:32], v[32:]]).astype(
            np.float32).reshape(P, 1)

    cb = np.concatenate([b21, b22, b21, b22]).astype(np.float32).reshape(P, 1)
    return cb, packv(cc[:, 0]), packv(cc[:, 1])


def _const_inputs(inputs):
    bf = ml_dtypes.bfloat16
    W1 = np.asarray(inputs["W1"], np.float32)     # [128, 10]
    b1 = np.asarray(inputs["b1"], np.float32)
    W21 = np.asarray(inputs["W21"], np.float32)   # [32, 128]
    b21 = np.asarray(inputs["b21"], np.float32)
    W22 = np.asarray(inputs["W22"], np.float32)
    b22 = np.asarray(inputs["b22"], np.float32)
    W31 = np.asarray(inputs["W31"], np.float32)   # [2, 32]
    b31 = np.asarray(inputs["b31"], np.float32)
    W32 = np.asarray(inputs["W32"], np.float32)   # [1, 32]
    b32 = np.asarray(inputs["b32"], np.float32)

    w1t = np.zeros((P, 128), np.float32)
    for s in range(4):
        w1t[32 * s:32 * s + F, :] = W1.T
    w2t = np.zeros((P, 64), np.float32)
    w2t[:, 0:32] = W21.T
    w2t[:, 32:64] = W22.T
    w3 = np.zeros((P, 4), np.float32)
    w3[0:32, 0:2] = W31.T
    w3[32:64, 2] = W32[0, :]
    w3[64:96, 0:2] = W31.T
    w3[96:128, 2] = W32[0, :]
    cb, c1v, c3v = _fit_l2_cubic(W1, b1, W21, b21, W22, b22)
    bav = np.zeros((P, 1), np.float32)
    for g in range(4):
        bav[32 * g + 0, 0] = b31[0]
        bav[32 * g + 1, 0] = b31[1]
        bav[32 * g + 2, 0] = b32[0]
    idb = np.eye(128, dtype=np.float32)
    return {
        "w1t": w1t.astype(bf), "w2t": w2t.astype(bf), "w3": w3.astype(bf),
        "idb": idb.astype(bf),
        "b1v": b1.reshape(P, 1), "cbv": cb, "c1v": c1v, "c3v": c3v,
        "bav": bav,
    }


def kernel(**inputs):
    import time
    from concourse.bass_utils import run_bass_kernel_spmd
    obs = np.ascontiguousarray(np.asarray(inputs["obs"], np.float32))
    consts = _const_inputs(inputs)
    nc = _build()
    in_maps = []
    for c in range(NCORES):
        m = {"obs": obs[c * BC:(c + 1) * BC]}
        m.update(consts)
        in_maps.append(m)
    last_err = None
    for attempt in range(3):
        try:
            res = run_bass_kernel_spmd(nc, in_maps, core_ids=list(range(NCORES)))
            break
        except Exception as e:  # transient device/tunnel flakiness: retry
            last_err = e
            time.sleep(3.0)
    else:
        raise last_err
    out = np.concatenate([res.results[c]["out"] for c in range(NCORES)], axis=0)
    return out


# revision 16
# speedup vs baseline: 1.4850x; 1.4850x over previous
"""BarrierNet Trainium2 kernel: tiny MLP (10->128->{32,32}->{2,1}) + halfspace QP
projection over a 524288-row batch, data-parallel over 8 NeuronCores.

Layout strategy per core (65536 samples):
  - obs loaded [128p, 5120f] fp32 in 8 piece-tiles (partition p holds samples
    p*512..p*512+511); DVE casts each piece into a 32-col-padded bf16 copy
    (10 real features + 22 zeros per block).
  - "block" j (0..511) = samples {p*512+j}. PE transposes 4 blocks per matmul
    (lhsT = padded [128,128] slab vs bf16 identity) -> X^T strips at 32-aligned
    partitions; MLP runs feature-on-partition with bf16 matmuls (W stationary),
    SiLU on ACT (PSUM->SBUF, bias fused), row/col tile-packed.
  - L3 emits [3@32g, 512] (u0,u1,alpha_pre); PE re-transposes (bf16) back to
    batch-on-partition; QP epilogue runs on DVE in fp32 from the original fp32
    obs cols 6..9 (strided views), one tanh on ACT for the sigmoid (tanh is in
    the silu ACT table set -> no table switch).
  - Epilogue + output DMA run in 8 pieces overlapped with the chunk stream.
ACT (12.6M SiLU elements/core) and PE (@1.2GHz observed) are co-bottlenecks.
"""

import numpy as np
import ml_dtypes

B, F, H1, C = 524288, 10, 128, 2
NCORES = 8
BC = B // NCORES            # 65536 samples per core
P = 128
CPP = BC // P               # 512 samples per partition == blocks per core
NBLK = CPP
BLK_PER_CHUNK = 16
NCHUNK = NBLK // BLK_PER_CHUNK   # 32
R2 = 0.8 * 0.8
NPC = 8                     # obs pieces == epilogue pieces
BPP = NBLK // NPC           # 64 blocks per piece
FP = 32                     # padded feature stride

_BUILT = None


def _legalize_single_wait(nc, mybir):
    """This walrus build encodes at most ONE sync wait per instruction; split
    multi-wait instructions into preceding NoOp wait-carriers."""
    n = 0
    for f in nc.m.functions:
        for b in f.blocks:
            new_list = []
            changed = False
            for inst in b.instructions:
                si = inst.sync_info
                if si is not None and len(si.on_wait) > 1:
                    waits = list(si.on_wait)
                    for k, w in enumerate(waits[1:]):
                        new_list.append(mybir.InstNoOp(
                            name=f"{inst.name}-wsplit-{k}", engine=inst.engine,
                            ins=[], outs=[],
                            sync_info=mybir.SyncInfo(on_update=[], on_wait=[w])))
                        n += 1
                    si.on_wait = waits[:1]
                    inst.sync_info = si
                    changed = True
                new_list.append(inst)
            if changed:
                b.instructions = new_list
    return n


def _build():
    global _BUILT
    if _BUILT is not None:
        return _BUILT
    import concourse.bass as bass
    import concourse.tile as tile
    import concourse.mybir as mybir

    f32 = mybir.dt.float32
    bf16 = mybir.dt.bfloat16
    AF = mybir.ActivationFunctionType
    ALU = mybir.AluOpType

    nc = bass.Bass("TRN2")
    obs_d = nc.dram_tensor("obs", [BC, F], f32, kind="ExternalInput")
    w1t_d = nc.dram_tensor("w1t", [P, 128], bf16, kind="ExternalInput")
    w2t_d = nc.dram_tensor("w2t", [P, 64], bf16, kind="ExternalInput")
    w3_d = nc.dram_tensor("w3", [P, 4], bf16, kind="ExternalInput")
    idb_d = nc.dram_tensor("idb", [P, 128], bf16, kind="ExternalInput")
    b1_d = nc.dram_tensor("b1v", [P, 1], f32, kind="ExternalInput")
    b2_d = nc.dram_tensor("b2v", [P, 1], f32, kind="ExternalInput")
    b3_d = nc.dram_tensor("b3v", [P, 1], f32, kind="ExternalInput")
    out_d = nc.dram_tensor("out", [BC, C], f32, kind="ExternalOutput")

    obs_ap = obs_d[:].rearrange("(p c) f -> p (c f)", p=P)   # [128, 5120]
    out_ap = out_d[:].rearrange("(p c) u -> p (c u)", p=P)   # [128, 1024]

    with tile.TileContext(nc) as tc:
        from contextlib import ExitStack
        es = ExitStack()
        with es:
            cpool = es.enter_context(tc.tile_pool(name="const", bufs=1))
            bigpool = es.enter_context(tc.tile_pool(name="big", bufs=1))
            wpool = es.enter_context(tc.tile_pool(name="work", bufs=3))
            epool = es.enter_context(tc.tile_pool(name="epi", bufs=1))
            ppool = es.enter_context(tc.tile_pool(name="ps", bufs=1, space="PSUM"))
            hpool = es.enter_context(tc.tile_pool(name="psh", bufs=2, space="PSUM"))

            # ---- obs piece tiles (pad memsets on the idle GpSimd queue) ----
            obsf = []          # 8 x [128, 640] fp32
            obsb = []          # 8 x [128, 2048] bf16, 32-col padded blocks
            for t in range(NPC):
                of = bigpool.tile([P, BPP * F], f32, tag=f"obsf{t}",
                                  name=f"obsf{t}")
                ob = bigpool.tile([P, BPP * FP], bf16, tag=f"obsb{t}",
                                  name=f"obsb{t}")
                obv = ob[:].rearrange("p (c f) -> p c f", f=FP)
                nc.gpsimd.memset(obv[:, :, F:FP], 0.0)
                obsf.append(of)
                obsb.append(ob)

            def load_piece(t):
                nc.sync.dma_start(out=obsf[t], in_=obs_ap[:, BPP * F * t:
                                                          BPP * F * (t + 1)])

            load_piece(0)

            # ---- constants (SP ring, right behind the first obs piece) ----
            idb = cpool.tile([P, 128], bf16)
            nc.sync.dma_start(out=idb, in_=idb_d[:])
            w1t = cpool.tile([P, 128], bf16)
            nc.sync.dma_start(out=w1t, in_=w1t_d[:])
            b1s = cpool.tile([P, 1], f32)
            nc.sync.dma_start(out=b1s, in_=b1_d[:])
            w2t = cpool.tile([P, 64], bf16)
            nc.sync.dma_start(out=w2t, in_=w2t_d[:])
            b2s = cpool.tile([P, 1], f32)
            nc.sync.dma_start(out=b2s, in_=b2_d[:])
            w3s = cpool.tile([P, 4], bf16)
            nc.sync.dma_start(out=w3s, in_=w3_d[:])
            b3s = cpool.tile([P, 1], f32)
            nc.sync.dma_start(out=b3s, in_=b3_d[:])

            for t in range(1, NPC):
                load_piece(t)

            ubuf = bigpool.tile([P, CPP * 3], f32)        # (u0,u1,apre)
            outt = bigpool.tile([P, CPP * C], f32)
            u0v = ubuf[:, 0::3]
            u1v = ubuf[:, 1::3]
            apv = ubuf[:, 2::3]

            def et(tag):
                return epool.tile([P, CPP], f32, tag=tag, name=tag)

            ss_t, t0_t, ggc_t, rec_t, b1p_t, e1_t, t2_t = (
                et("ss"), et("t0"), et("ggc"), et("rec"), et("b1p"), et("e1"),
                et("t2"))
            th_t, d0_t, c0_t, d1_t, c1_t, cc_t, ff_t, mn_t, q2_t, s0_t, s1_t = (
                et("th"), et("d0"), et("c0"), et("d1"), et("c1"), et("cc"),
                et("ff"), et("mn"), et("q2"), et("s0"), et("s1"))

            def epi(t):
                """QP epilogue + store for piece t (blocks [64t, 64t+64))."""
                c0, c1 = BPP * t, BPP * (t + 1)
                sl = slice(c0, c1)
                of = obsf[t]
                rx, ry = of[:, 6::F], of[:, 7::F]
                vx, vy = of[:, 8::F], of[:, 9::F]
                ss, t0, ggc, rec, b1p, e1, t2 = (
                    x[:, sl] for x in (ss_t, t0_t, ggc_t, rec_t, b1p_t, e1_t,
                                       t2_t))
                th, d0, c0_, d1, c1_, cc, ff, mn, q2, s0, s1 = (
                    x[:, sl] for x in (th_t, d0_t, c0_t, d1_t, c1_t, cc_t,
                                       ff_t, mn_t, q2_t, s0_t, s1_t))
                u0, u1, ap_ = u0v[:, sl], u1v[:, sl], apv[:, sl]
                TT = nc.vector.tensor_tensor
                TT(out=ss, in0=rx, in1=rx, op=ALU.mult)
                TT(out=t0, in0=ry, in1=ry, op=ALU.mult)
                TT(out=ss, in0=ss, in1=t0, op=ALU.add)
                nc.vector.tensor_scalar(out=ggc, in0=ss, scalar1=4.0,
                                        scalar2=1e-12, op0=ALU.mult, op1=ALU.max)
                nc.vector.reciprocal(out=rec, in_=ggc)
                nc.vector.tensor_single_scalar(out=b1p, in_=ss, scalar=R2,
                                               op=ALU.subtract)
                nc.scalar.activation(out=th, in_=ap_, func=AF.Tanh, scale=0.5)
                TT(out=e1, in0=th, in1=b1p, op=ALU.mult)
                TT(out=t2, in0=b1p, in1=e1, op=ALU.add)
                TT(out=d0, in0=u0, in1=vx, op=ALU.subtract)
                TT(out=c0_, in0=rx, in1=d0, op=ALU.mult)
                TT(out=d1, in0=u1, in1=vy, op=ALU.subtract)
                TT(out=c1_, in0=ry, in1=d1, op=ALU.mult)
                TT(out=cc, in0=c0_, in1=c1_, op=ALU.add)
                TT(out=ff, in0=cc, in1=t2, op=ALU.add)
                nc.vector.tensor_single_scalar(out=mn, in_=ff, scalar=0.0,
                                               op=ALU.min)
                TT(out=q2, in0=mn, in1=rec, op=ALU.mult)
                TT(out=s0, in0=q2, in1=rx, op=ALU.mult)
                nc.vector.scalar_tensor_tensor(out=outt[:, 0::2][:, sl], in0=s0,
                                               scalar=-4.0, in1=u0,
                                               op0=ALU.mult, op1=ALU.add)
                TT(out=s1, in0=q2, in1=ry, op=ALU.mult)
                nc.vector.scalar_tensor_tensor(out=outt[:, 1::2][:, sl], in0=s1,
                                               scalar=-4.0, in1=u1,
                                               op0=ALU.mult, op1=ALU.add)
                nc.sync.dma_start(out=out_ap[:, 2 * c0:2 * c1],
                                  in_=outt[:, 2 * c0:2 * c1])

            # ---- main chunk loop (3-stage software pipeline) ----
            # A1(q): obs cast + PE transpose + X^T copy   (chunk q)
            # A2(q): L1 matmuls + SiLU                    (one chunk behind)
            # B(q):  L2/SiLU/L3/back-transpose/extract    (two chunks behind)
            def stageA1(q):
                ob = obsb[q // (NCHUNK // NPC)]
                qloc = q % (NCHUNK // NPC)
                if qloc == 0:
                    t = q // (NCHUNK // NPC)
                    obv = ob[:].rearrange("p (c f) -> p c f", f=FP)
                    nc.vector.tensor_copy(
                        out=obv[:, :, 0:F],
                        in_=obsf[t][:].rearrange("p (c f) -> p c f", f=F))
                xtp = ppool.tile([P, 512], f32, tag="xt", name="xtp")
                for jj in range(BLK_PER_CHUNK):
                    s, m = jj % 4, jj // 4
                    jloc = qloc * 16 + 4 * m + s
                    nc.tensor.matmul(
                        out=xtp[32 * s:32 * s + 10, 128 * m:128 * m + 128],
                        lhsT=ob[:, jloc * FP:jloc * FP + F],
                        rhs=idb[:],
                        tile_position=(0, 32 * s),
                    )
                xts = wpool.tile([P, 512], bf16, tag="xts", name="xts")
                nc.vector.tensor_copy(out=xts[:], in_=xtp[:])
                return xts, xtp

            def stageA2(q, xts):
                l1p = ppool.tile([P, 2048], f32, tag="l1", name="l1p")
                for s in range(4):
                    nc.tensor.matmul(
                        out=l1p[:, 512 * s:512 * (s + 1)],
                        lhsT=w1t[32 * s:32 * s + 10, :],
                        rhs=xts[32 * s:32 * s + 10, :],
                        tile_position=(32 * s, 0),
                    )
                h1 = wpool.tile([P, 2048], bf16, tag="h1", name="h1")
                nc.scalar.activation(out=h1[:], in_=l1p[:], func=AF.Silu,
                                     bias=b1s[:, 0:1], scale=1.0)
                return h1

            def stageB(q, h1):
                l2p = ppool.tile([P, 1024], f32, tag="l2", name="l2p")
                for g in range(4):
                    pb = 64 * (g % 2)
                    nc.tensor.matmul(
                        out=l2p[pb:pb + 64, 512 * (g // 2):512 * (g // 2) + 512],
                        lhsT=w2t[:],
                        rhs=h1[:, 512 * g:512 * (g + 1)],
                        tile_position=(0, pb),
                    )
                x2 = wpool.tile([P, 1024], bf16, tag="x2", name="x2")
                nc.scalar.activation(out=x2[:], in_=l2p[:], func=AF.Silu,
                                     bias=b2s[:, 0:1], scale=1.0)
                u3p = ppool.tile([P, 512], f32, tag="u3", name="u3p")
                for g in range(4):
                    pb = 64 * (g % 2)
                    nc.tensor.matmul(
                        out=u3p[32 * g:32 * g + 3, :],
                        lhsT=w3s[pb:pb + 64, 0:3],
                        rhs=x2[pb:pb + 64, 512 * (g // 2):512 * (g // 2) + 512],
                        tile_position=(pb, 32 * g),
                    )
                u3s = wpool.tile([P, 512], bf16, tag="u3s", name="u3s")
                nc.vector.tensor_scalar_add(out=u3s[:], in0=u3p[:],
                                            scalar1=b3s[:, 0:1])
                utp = ppool.tile([P, 512], bf16, tag="u3", name="utp")
                for b in range(4):
                    nc.tensor.transpose(
                        out=utp[:, 128 * b:128 * (b + 1)],
                        in_=u3s[:, 128 * b:128 * (b + 1)],
                        identity=idb[:],
                    )
                src = utp[:].rearrange("p (b g x) -> p b g x", b=4, g=4)[:, :, :, 0:3]
                dst = ubuf[:, 48 * q:48 * (q + 1)].rearrange(
                    "p (b g f) -> p b g f", b=4, g=4)
                nc.vector.tensor_copy(out=dst, in_=src)
                if (q + 1) % (NCHUNK // NPC) == 0:
                    epi((q + 1) // (NCHUNK // NPC) - 1)

            xts_d, h1_d = {}, {}
            for i in range(NCHUNK + 2):
                if i < NCHUNK:
                    xts_d[i] = stageA1(i)
                if 1 <= i <= NCHUNK:
                    xts_p, _ = xts_d.pop(i - 1)
                    h1_d[i - 1] = stageA2(i - 1, xts_p)
                if i >= 2:
                    stageB(i - 2, h1_d.pop(i - 2))

    _legalize_single_wait(nc, mybir)
    _BUILT = nc
    return nc


def _const_inputs(inputs):
    bf = ml_dtypes.bfloat16
    W1 = np.asarray(inputs["W1"], np.float32)     # [128, 10]
    b1 = np.asarray(inputs["b1"], np.float32)     # [128]
    W21 = np.asarray(inputs["W21"], np.float32)   # [32, 128]
    b21 = np.asarray(inputs["b21"], np.float32)
    W22 = np.asarray(inputs["W22"], np.float32)
    b22 = np.asarray(inputs["b22"], np.float32)
    W31 = np.asarray(inputs["W31"], np.float32)   # [2, 32]
    b31 = np.asarray(inputs["b31"], np.float32)
    W32 = np.asarray(inputs["W32"], np.float32)   # [1, 32]
    b32 = np.asarray(inputs["b32"], np.float32)

    w1t = np.zeros((P, 128), np.float32)
    for s in range(4):
        w1t[32 * s:32 * s + 10, :] = W1.T
    w2t = np.zeros((P, 64), np.float32)
    w2t[:, 0:32] = W21.T
    w2t[:, 32:64] = W22.T
    w3 = np.zeros((P, 4), np.float32)
    w3[0:32, 0:2] = W31.T
    w3[32:64, 2] = W32[0, :]
    w3[64:96, 0:2] = W31.T
    w3[96:128, 2] = W32[0, :]
    b1v = b1.reshape(P, 1)
    b2v = np.concatenate([b21, b22, b21, b22]).reshape(P, 1)
    b3 = np.array([b31[0], b31[1], b32[0]], np.float32)
    b3v = np.zeros((P, 1), np.float32)
    for g in range(4):
        b3v[32 * g:32 * g + 3, 0] = b3
    idb = np.eye(128, dtype=np.float32)
    return {
        "w1t": w1t.astype(bf), "w2t": w2t.astype(bf), "w3": w3.astype(bf),
        "idb": idb.astype(bf),
        "b1v": b1v, "b2v": b2v, "b3v": b3v,
    }


def kernel(**inputs):
    import time
    from concourse.bass_utils import run_bass_kernel_spmd
    obs = np.ascontiguousarray(np.asarray(inputs["obs"], np.float32))
    nc = _build()
    consts = _const_inputs(inputs)
    in_maps = []
    for c in range(NCORES):
        m = {"obs": obs[c * BC:(c + 1) * BC]}
        m.update(consts)
        in_maps.append(m)
    last_err = None
    for attempt in range(3):
        try:
            res = run_bass_kernel_spmd(nc, in_maps, core_ids=list(range(NCORES)))
            break
        except Exception as e:  # transient device/tunnel flakiness: retry
            last_err = e
            time.sleep(3.0)
    else:
        raise last_err
    out = np.concatenate([res.results[c]["out"] for c in range(NCORES)], axis=0)
    return out

